# revision 3
# baseline (speedup 1.0000x reference)
"""Distributed Bass kernel for causal multi-head attention on 8 TRN2 NeuronCores.

Problem: B=2, S=2048, D=1024, H=16 (dh=64) causal attention layer.
Sharding: core c = (batch b = c//4, head-group g = c%4 covering 4 heads).

Communication: this environment only supports full-8-rank AllReduce reliably
(AllGather / ReduceScatter / subgroup collectives / dynamic-offset DMA /
custom gpsimd-DVE ops all hang), so the output projection is computed as a
LOCAL partial from each core's 4 heads into a [2*1024, S] transposed-output
buffer (row = batch*1024 + oc), using per-core Wo inputs that are ZERO for
the other batch's block — the AllReduce sum then assembles the exact output
with no SPMD-divergent addressing anywhere. Two ARs (output-column halves)
so the first overlaps the second half of the projection. Every core gets the
full summed out^T; the host takes each core's slice and transposes.

Device-side notes:
  - Host passes states pre-transposed ([D, S] f32) so every matmul has the
    contraction dim on partitions; no on-device transposes.
  - All matmul operands bf16 (f32 PSUM accumulation); f32->bf16 casts happen
    inside SWDGE DMA (gpsimd), costing no engine time.
  - Scores computed transposed [k, q]: softmax-weighted ctx needs no alpha
    transpose; softmax denominator via a ones-column appended to V (M=65);
    normalization = K=1 ones-matmul broadcast + plain DVE reciprocal.
  - No max-subtraction in softmax (scores ~N(0,1); exp is safe).
  - dh=64 head pairs packed into the PE array via base partitions 0/64.
  - Causality: k-blocks above the diagonal skipped; diagonal blocks get a
    multiplicative 0/1 mask (4 precomputed alignment tiles, extra input).
"""

import numpy as np

import concourse.bass as bass
import concourse.bacc as bacc
import concourse.mybir as mybir
import concourse.tile as tile
from concourse import bass_utils

F32 = mybir.dt.float32
BF16 = mybir.dt.bfloat16
EXP = mybir.ActivationFunctionType.Exp

B, S, D, H = 2, 2048, 1024, 16
DH = 64            # head dim
HG = 4             # heads per core (head group)
NP = 2             # head pairs per core
QC = 512           # q-chunk (matmul moving N)
NJ = S // QC       # 4 q-chunks
KB = 128           # k block (partition tile)
NKB = S // KB      # 16 k blocks
DB = D // 128      # 8 contraction blocks of 128
NCORE = 8


def _build():
    nc = bacc.Bacc(
        "TRN2", target_bir_lowering=False, debug=False,
        enable_asserts=False, num_devices=NCORE,
    )

    statesT = nc.dram_tensor("statesT", [D, S], F32, kind="ExternalInput")
    wq_d = nc.dram_tensor("wq", [D, 256], F32, kind="ExternalInput")
    wk_d = nc.dram_tensor("wk", [D, 256], F32, kind="ExternalInput")
    wv_d = nc.dram_tensor("wv", [D, 256], F32, kind="ExternalInput")
    # wo: [64, (bb*4 + h)*8 + ob)*128 + j] stationary slices, zero for bb != b
    wo_d = nc.dram_tensor("wo", [64, 2 * HG * 8 * 128], F32, kind="ExternalInput")
    cm_d = nc.dram_tensor("cmask", [128, 4 * QC], F32, kind="ExternalInput")
    out_d = nc.dram_tensor("out", [2 * 1024, S], BF16, kind="ExternalOutput")

    with tile.TileContext(nc) as tc:
        with (
            tc.tile_pool(name="const", bufs=1) as constp,
            tc.tile_pool(name="alpha", bufs=3) as alphap,
            tc.tile_pool(name="nrm", bufs=2) as nrmp,
            tc.tile_pool(name="stg", bufs=4) as stgp,
            tc.tile_pool(name="ps", bufs=2, space="PSUM") as psp,
            tc.tile_pool(name="psc", bufs=2, space="PSUM") as pscp,
            tc.tile_pool(name="psb", bufs=2, space="PSUM") as psbp,
            tc.tile_pool(name="dram", bufs=1, space="DRAM") as dramp,
        ):
            # ---------------- persistent SBUF tensors ----------------
            st = constp.tile([128, DB * S], BF16, tag="st")        # statesT: [d%128, db*S + s]
            wq = constp.tile([128, DB * 256], BF16, tag="wq")      # [d%128, db*256 + c]
            wk = constp.tile([128, DB * 256], BF16, tag="wk")
            wv = constp.tile([128, DB * 256], BF16, tag="wv")
            wo = constp.tile([64, 2 * HG * 8 * 128], BF16, tag="wo")
            cm = constp.tile([128, 4 * QC], BF16, tag="cm")        # 4 causal mask alignments
            qt = constp.tile([128, NP * S], BF16, tag="qt")        # [pair-local c, p*S + q]
            kt = constp.tile([128, NP * S], BF16, tag="kt")
            vp = constp.tile([128, NKB * (HG * 65)], BF16, tag="vp")  # V' 65-pitch + ones col
            ctxu = constp.tile([64, HG * S], BF16, tag="ctxu")     # normalized ctx^T per head
            ones = constp.tile([65, 64], F32, tag="ones")          # row 64: K=1 bcast matmul

            # -------- loads (SWDGE cast f32 -> bf16 during DMA) --------
            st_view = statesT.ap().rearrange("(a p) s -> p a s", p=128)
            for db in range(DB):  # chunked so QKV matmuls can start early
                nc.gpsimd.dma_start(
                    st[:, db * S:(db + 1) * S], st_view[:, db, :])
            for w_sb, w_dr in ((wq, wq_d), (wk, wk_d), (wv, wv_d)):
                nc.gpsimd.dma_start(
                    w_sb[:].rearrange("p (a c) -> p a c", a=DB),
                    w_dr.ap().rearrange("(a p) c -> p a c", p=128))
            nc.gpsimd.dma_start(wo[:], wo_d[:, :])
            nc.gpsimd.dma_start(cm[:], cm_d[:, :])

            nc.vector.memset(ones[64:65, :], 1.0)
            # V' ones columns (denominator trick)
            nc.vector.memset(
                vp[:].rearrange("p (n w) -> p n w", w=65)[:, :, 64:65], 1.0)

            # ---------------- QKV projections ----------------
            for dst, w_sb in ((qt, wq), (kt, wk)):
                for p in range(NP):
                    for jj in range(2):  # 1024 q-cols per psum tile
                        ps = psp.tile([128, 1024], F32, tag="ps",
                                      name=f"qk{p}_{jj}")
                        for half in range(2):
                            q0 = jj * 1024 + half * QC
                            for db in range(DB):
                                nc.tensor.matmul(
                                    ps[:, half * QC:(half + 1) * QC],
                                    w_sb[:, db * 256 + p * 128: db * 256 + (p + 1) * 128],
                                    st[:, db * S + q0: db * S + q0 + QC],
                                    start=(db == 0), stop=(db == DB - 1))
                        nc.any.tensor_copy(
                            dst[:, p * S + jj * 1024: p * S + (jj + 1) * 1024], ps[:])

            # V in [s, c] layout, written into 65-pitch V' slots
            for kb in range(NKB):
                ps = psp.tile([128, 1024], F32, tag="ps", name=f"v{kb}")
                for db in range(DB):
                    nc.tensor.matmul(
                        ps[:, 0:256],
                        st[:, db * S + kb * KB: db * S + (kb + 1) * KB],
                        wv[:, db * 256:(db + 1) * 256],
                        start=(db == 0), stop=(db == DB - 1))
                nc.any.tensor_copy(
                    vp[:, kb * HG * 65:(kb + 1) * HG * 65]
                      .rearrange("p (h w) -> p h w", w=65)[:, :, 0:64],
                    ps[:, 0:256].rearrange("p (h w) -> p h w", w=64))

            # ---- attention (j-outer so per-q-chunk ARs fire early) ----
            cc_chunks = []
            for j in range(NJ):
                for p in range(NP):
                    cx = [pscp.tile([65, QC], F32, tag="psc",
                                    name=f"cx{p}_{j}_{hi}") for hi in range(2)]
                    nkb = 4 * j + 4
                    for kb in range(nkb):
                        ps = psp.tile([128, 1024], F32, tag="ps",
                                      name=f"s{p}_{j}_{kb}")
                        for hi in range(2):
                            h0 = hi * 64
                            nc.tensor.matmul(
                                ps[:, hi * QC:(hi + 1) * QC],
                                kt[h0:h0 + 64, p * S + kb * KB: p * S + (kb + 1) * KB],
                                qt[h0:h0 + 64, p * S + j * QC: p * S + (j + 1) * QC],
                                start=True, stop=True)
                        al = alphap.tile([128, 1024], BF16, tag="alpha",
                                         name=f"al{p}_{j}_{kb}")
                        nc.scalar.activation(al[:], ps[:], EXP, scale=0.125)
                        m = kb - 4 * j
                        if m >= 0:  # diagonal block: causal mask (multiplicative)
                            for hi in range(2):
                                nc.vector.tensor_mul(
                                    al[:, hi * QC:(hi + 1) * QC],
                                    al[:, hi * QC:(hi + 1) * QC],
                                    cm[:, m * QC:(m + 1) * QC])
                        for hi in range(2):
                            h = 2 * p + hi
                            nc.tensor.matmul(
                                cx[hi][:, :],
                                vp[:, kb * HG * 65 + h * 65: kb * HG * 65 + (h + 1) * 65],
                                al[:, hi * QC:(hi + 1) * QC],
                                start=(kb == 0), stop=(kb == nkb - 1))
                    # normalize: ctx[c, q] * (1 / denom[q]); denom = psum row 64.
                    # Vanilla ops only: K=1 ones-matmul broadcasts the denom row
                    # across 64 partitions, then DVE reciprocal + multiply.
                    for hi in range(2):
                        h = 2 * p + hi
                        dn = nrmp.tile([65, QC], F32, tag="dn",
                                       name=f"dn{p}_{j}_{hi}")
                        nc.vector.tensor_copy(dn[64:65, :], cx[hi][64:65, :])
                        pb = psbp.tile([64, QC], F32, tag="psb",
                                       name=f"pb{p}_{j}_{hi}")
                        nc.tensor.matmul(pb[:], ones[64:65, :], dn[64:65, :],
                                         start=True, stop=True)
                        db_ = nrmp.tile([64, QC], F32, tag="db",
                                        name=f"db{p}_{j}_{hi}")
                        nc.vector.tensor_copy(db_[:], pb[:])
                        rb = nrmp.tile([64, QC], F32, tag="rb",
                                       name=f"rb{p}_{j}_{hi}")
                        nc.vector.reciprocal(rb[:], db_[:])
                        nc.vector.tensor_mul(
                            ctxu[:, h * S + j * QC: h * S + (j + 1) * QC],
                            cx[hi][0:64, :], rb[:])

                # ---- q-chunk j complete for all heads: project + AllReduce ----
                # partial^T[bb*1024 + ob*128 + oc, j-cols] = sum_h ctx_h^T . wo
                # wo is zero for bb != this core's batch, so the 8-rank AR sum
                # assembles the exact output; overlaps later chunks' attention.
                cc_in = dramp.tile([2048, QC], BF16, tag=f"cci{j}",
                                   name=f"cci{j}")
                for ob in range(8):
                    for bb in range(2):
                        ps = psp.tile([128, 1024], F32, tag="ps",
                                      name=f"o{j}_{ob}_{bb}")
                        for h in range(HG):
                            nc.tensor.matmul(
                                ps[:, 0:QC],
                                wo[:, ((bb * HG + h) * 8 + ob) * 128:
                                      ((bb * HG + h) * 8 + ob + 1) * 128],
                                ctxu[:, h * S + j * QC: h * S + (j + 1) * QC],
                                start=(h == 0), stop=(h == HG - 1))
                        stage = stgp.tile([128, QC], BF16, tag="stage",
                                          name=f"stg{j}_{ob}_{bb}")
                        nc.any.tensor_copy(stage[:], ps[:, 0:QC])
                        nc.sync.dma_start(
                            cc_in[bb * 1024 + ob * 128: bb * 1024 + (ob + 1) * 128, :],
                            stage[:])
                cc_out = dramp.tile([2048, QC], BF16, tag=f"cco{j}",
                                    name=f"cco{j}")
                nc.gpsimd.collective_compute(
                    "AllReduce", mybir.AluOpType.add,
                    replica_groups=[list(range(NCORE))],
                    ins=[cc_in[:].opt()], outs=[cc_out[:].opt()])
                cc_chunks.append(cc_out)
                nc.gpsimd.dma_start(
                    out_d[:, j * QC:(j + 1) * QC], cc_out[:])

    nc.compile()
    return nc


_NC = None
_LAST_RESULTS = None  # BassKernelResults of the most recent run (for test harness)


def _causal_mask_tiles() -> np.ndarray:
    r = np.arange(128)[:, None]
    col = np.arange(QC)[None, :]
    tiles = [(col >= r + 128 * m).astype(np.float32) for m in range(4)]
    return np.concatenate(tiles, axis=1)  # [128, 2048]


def _wo_input(Wo: np.ndarray, b: int, g: int) -> np.ndarray:
    """Per-core Wo stationary slices: [64, ((bb*4 + h)*8 + ob)*128 + j] with
    wo[d, (bb,h,ob,j)] = Wo[(4g + h)*64 + d, ob*128 + j] if bb == b else 0."""
    wo_in = np.zeros((64, 2 * HG * 8 * 128), np.float32)
    for h in range(HG):
        blk = Wo[(4 * g + h) * DH:(4 * g + h + 1) * DH, :]  # [64, 1024]
        base = (b * HG + h) * 8 * 128
        wo_in[:, base:base + 1024] = blk
    return wo_in


def kernel(states, masks, Wq, Wk, Wv, Wo):
    global _NC
    if _NC is None:
        _NC = _build()
    states = np.asarray(states, np.float32)
    Wq, Wk, Wv, Wo = (np.asarray(w, np.float32) for w in (Wq, Wk, Wv, Wo))
    cm = _causal_mask_tiles()

    in_maps = []
    for c in range(NCORE):
        b, g = c // 4, c % 4
        cs = slice(g * 256, (g + 1) * 256)
        in_maps.append({
            "statesT": np.ascontiguousarray(states[b].T),
            "wq": np.ascontiguousarray(Wq[:, cs]),
            "wk": np.ascontiguousarray(Wk[:, cs]),
            "wv": np.ascontiguousarray(Wv[:, cs]),
            "wo": _wo_input(Wo, b, g),
            "cmask": cm,
        })

    res = bass_utils.run_bass_kernel_spmd(_NC, in_maps, core_ids=list(range(NCORE)))
    global _LAST_RESULTS
    _LAST_RESULTS = res
    # all cores hold the identical AllReduced out^T; row = bb*1024 + oc
    full = np.asarray(res.results[0]["out"]).astype(np.float32)  # [2048, S]
    out = np.empty((B, S, D), np.float32)
    for bb in range(B):
        out[bb] = full[bb * 1024:(bb + 1) * 1024, :].T
    return out



# revision 8
# speedup vs baseline: 1.2973x; 1.2973x over previous
"""Distributed Bass kernel for causal multi-head attention on 8 TRN2 NeuronCores.

Problem: B=2, S=2048, D=1024, H=16 (dh=64) causal attention layer.
Sharding: core c = (batch b = c//4, head-group g = c%4 covering 4 heads).

Communication (v2): instead of an 8-rank AllReduce of half-zero-padded
partials (2x wire traffic + zero-half matmuls), the output projection
partials are exchanged with a single 8-rank AllToAll per q-chunk and
reduced locally:
  - Output-row ownership: core d owns out^T rows [d*128,(d+1)*128) of BOTH
    batches. Each core's chunk partial [1024 own-batch rows, 512 q] is
    sliced into 8 destination shards of [128, 512].
  - SPMD-uniform batch placement: each A2A shard is [2 batch-halves][128]
    [512]; the staging multiply scales the partial by per-core host flags
    (bsel = [1,0] or [0,1]), so the own-batch half carries the data and the
    other half carries zeros -- no divergent addressing anywhere.
  - After the A2A, 7 elementwise adds (gpsimd, off the DVE critical path)
    sum the 8 source shards; zeros from other-batch sources are harmless.
  - Each core writes only its owned [128, 2*2048] slice; the host stitches
    slices from all 8 cores (no broadcast needed).
  Wire per chunk: 7/8 * 2MB vs AllReduce's ~2 * 2MB, and no zero-half
  output projection matmuls.

Device-side notes:
  - All inputs are pre-cast to bf16 on the host: input DMA volume halves
    and loads run on the HWDGE sync queue (no SWDGE cast).
  - Scores computed transposed [k, q]: softmax-weighted ctx needs no alpha
    transpose; softmax denominator via a ones-column appended to V (M=65).
  - No max-subtraction in softmax (scores ~N(0,1); exp is safe).
  - Causality: k-blocks above the diagonal skipped; diagonal blocks are
    column-trimmed (matmul/exp/mask/ctx restricted to cols >= m*128) and
    get a multiplicative 0/1 mask.
  - Normalization: reciprocal on the [1,512] denominator rows, one psum
    [128,512] broadcast via two accumulated outer-product matmuls, one
    psum->sbuf copy, two muls into the pair-stacked ctxu2.
  - ctxu2 stacks head pairs on partitions (head 2p+hi at partitions
    [hi*64,(hi+1)*64)), so output projection contracts K=128 (2 heads per
    matmul) -- half the matmuls of the K=64 variant.
"""

import numpy as np
import ml_dtypes

import concourse.bass as bass
import concourse.bacc as bacc
import concourse.mybir as mybir
import concourse.tile as tile
from concourse import bass_utils

DEBUG_DUMPS = False

F32 = mybir.dt.float32
BF16 = mybir.dt.bfloat16
EXP = mybir.ActivationFunctionType.Exp

B, S, D, H = 2, 2048, 1024, 16
DH = 64            # head dim
HG = 4             # heads per core (head group)
NP = 2             # head pairs per core
QC = 512           # q-chunk
NJ = S // QC       # 4 q-chunks
KB = 128           # k block (partition tile)
NKB = S // KB      # 16 k blocks
DB = D // 128      # 8 contraction blocks of 128
NCORE = 8


def _build():
    nc = bacc.Bacc(
        "TRN2", target_bir_lowering=False, debug=False,
        enable_asserts=False, num_devices=NCORE,
    )

    statesT = nc.dram_tensor("statesT", [D, S], BF16, kind="ExternalInput")
    wq_d = nc.dram_tensor("wq", [D, 256], BF16, kind="ExternalInput")
    wk_d = nc.dram_tensor("wk", [D, 256], BF16, kind="ExternalInput")
    wv_d = nc.dram_tensor("wv", [D, 256], BF16, kind="ExternalInput")
    # wo2[r, (pp*8 + ob)*128 + c] = Wo[(4g + 2pp + r//64)*64 + r%64, ob*128 + c]
    wo2_d = nc.dram_tensor("wo2", [128, NP * 8 * 128], BF16, kind="ExternalInput")
    cm_d = nc.dram_tensor("cmask", [128, 4 * QC], BF16, kind="ExternalInput")
    bsel_d = nc.dram_tensor("bsel", [128, 2], F32, kind="ExternalInput")
    # out: core-owned slice; [r, b*2048 + s] = out^T[own_row_base + r, s] for batch b
    out_d = nc.dram_tensor("out", [128, B * S], BF16, kind="ExternalOutput")
    if DEBUG_DUMPS:
        dbg_qt = nc.dram_tensor("dbg_qt", [128, NP * S], BF16, kind="ExternalOutput")
        dbg_kt = nc.dram_tensor("dbg_kt", [128, NP * S], BF16, kind="ExternalOutput")
        dbg_ctxu = nc.dram_tensor("dbg_ctxu", [128, NP * S], BF16, kind="ExternalOutput")
        dbg_ccin = nc.dram_tensor("dbg_ccin", [2048, QC], BF16, kind="ExternalOutput")
        dbg_ccout = nc.dram_tensor("dbg_ccout", [2048, QC], BF16, kind="ExternalOutput")

    with tile.TileContext(nc) as tc:
        with (
            tc.tile_pool(name="const", bufs=1) as constp,
            tc.tile_pool(name="alpha", bufs=3) as alphap,
            tc.tile_pool(name="nrm", bufs=2) as nrmp,
            tc.tile_pool(name="rbs", bufs=2) as rbsp,
            tc.tile_pool(name="stg", bufs=4) as stgp,
            tc.tile_pool(name="rb", bufs=2) as rbp,
            tc.tile_pool(name="sum", bufs=2) as sump,
            tc.tile_pool(name="ps", bufs=2, space="PSUM") as psp,
            tc.tile_pool(name="psc", bufs=2, space="PSUM") as pscp,
            tc.tile_pool(name="pso", bufs=2, space="PSUM") as psop,
            tc.tile_pool(name="dram", bufs=1, space="DRAM") as dramp,
        ):
            # ---------------- persistent SBUF tensors ----------------
            st = constp.tile([128, DB * S], BF16, tag="st")        # statesT: [d%128, db*S + s]
            wq = constp.tile([128, DB * 256], BF16, tag="wq")      # [d%128, db*256 + c]
            wk = constp.tile([128, DB * 256], BF16, tag="wk")
            wv = constp.tile([128, DB * 256], BF16, tag="wv")
            wo2 = constp.tile([128, NP * 8 * 128], BF16, tag="wo2")
            cm = constp.tile([128, 4 * QC], BF16, tag="cm")        # 4 causal mask alignments
            bsel = constp.tile([128, 2], F32, tag="bsel")
            qt = constp.tile([128, NP * S], BF16, tag="qt")        # [pair-local c, p*S + q]
            kt = constp.tile([128, NP * S], BF16, tag="kt")
            vp = constp.tile([128, NKB * (HG * 65)], BF16, tag="vp")  # V' 65-pitch + ones col
            ctxu2 = constp.tile([128, NP * S], BF16, tag="ctxu2")  # pair-stacked normalized ctx^T
            ones2 = constp.tile([65, 256], BF16, tag="ones2")       # bcast outer-product lhsTs

            # -------- loads (all bf16; HWDGE sync queue) --------
            st_view = statesT.ap().rearrange("(a p) s -> p a s", p=128)
            nc.sync.dma_start(
                wq[:].rearrange("p (a c) -> p a c", a=DB),
                wq_d.ap().rearrange("(a p) c -> p a c", p=128))
            for db in range(DB):  # chunked so QKV matmuls can start early
                nc.sync.dma_start(st[:, db * S:(db + 1) * S], st_view[:, db, :])
            for w_sb, w_dr in ((wk, wk_d), (wv, wv_d)):
                nc.sync.dma_start(
                    w_sb[:].rearrange("p (a c) -> p a c", a=DB),
                    w_dr.ap().rearrange("(a p) c -> p a c", p=128))
            nc.sync.dma_start(cm[:], cm_d[:, :])
            nc.sync.dma_start(wo2[:], wo2_d[:, :])
            nc.sync.dma_start(bsel[:], bsel_d[:, :])

            nc.vector.memset(ones2[64:65, :], 0.0)
            nc.vector.memset(ones2[64:65, 0:64], 1.0)     # h0: out partitions 0-63
            nc.vector.memset(ones2[64:65, 192:256], 1.0)  # h1: out partitions 64-127
            # V' ones columns (denominator trick)
            nc.vector.memset(
                vp[:].rearrange("p (n w) -> p n w", w=65)[:, :, 64:65], 1.0)

            # ---------------- QKV projections ----------------
            for dst, w_sb in ((qt, wq), (kt, wk)):
                for p in range(NP):
                    for jj in range(2):  # 1024 q-cols per psum tile
                        ps = psp.tile([128, 1024], F32, tag="ps",
                                      name=f"qk{p}_{jj}")
                        for half in range(2):
                            q0 = jj * 1024 + half * QC
                            for db in range(DB):
                                nc.tensor.matmul(
                                    ps[:, half * QC:(half + 1) * QC],
                                    w_sb[:, db * 256 + p * 128: db * 256 + (p + 1) * 128],
                                    st[:, db * S + q0: db * S + q0 + QC],
                                    start=(db == 0), stop=(db == DB - 1))
                        nc.any.tensor_copy(
                            dst[:, p * S + jj * 1024: p * S + (jj + 1) * 1024], ps[:])

            # V in [s, c] layout, written into 65-pitch V' slots
            for kb in range(NKB):
                psv = psop.tile([128, 512], F32, tag="pso", name=f"v{kb}")
                for db in range(DB):
                    nc.tensor.matmul(
                        psv[:, 0:256],
                        st[:, db * S + kb * KB: db * S + (kb + 1) * KB],
                        wv[:, db * 256:(db + 1) * 256],
                        start=(db == 0), stop=(db == DB - 1))
                nc.any.tensor_copy(
                    vp[:, kb * HG * 65:(kb + 1) * HG * 65]
                      .rearrange("p (h w) -> p h w", w=65)[:, :, 0:64],
                    psv[:, 0:256].rearrange("p (h w) -> p h w", w=64))

            # ---- attention (j-outer; A2A per q-chunk overlaps compute) ----
            cc_pairs = []
            for j in range(NJ):
                for p in range(NP):
                    cx = [pscp.tile([65, QC], F32, tag="psc",
                                    name=f"cx{p}_{j}_{hi}") for hi in range(2)]
                    nkb = 4 * j + 4
                    for kb in range(nkb):
                        m = kb - 4 * j
                        col0 = max(m, 0) * 128  # diagonal column trim
                        ps = psp.tile([128, 1024], F32, tag="ps",
                                      name=f"s{p}_{j}_{kb}")
                        for hi in range(2):
                            h0 = hi * 64
                            nc.tensor.matmul(
                                ps[:, hi * QC + col0:(hi + 1) * QC],
                                kt[h0:h0 + 64, p * S + kb * KB: p * S + (kb + 1) * KB],
                                qt[h0:h0 + 64, p * S + j * QC + col0: p * S + (j + 1) * QC],
                                start=True, stop=True)
                        al = alphap.tile([128, 1024], BF16, tag="alpha",
                                         name=f"al{p}_{j}_{kb}")
                        if m < 0:
                            nc.scalar.activation(al[:], ps[:], EXP, scale=0.125)
                        else:  # diagonal: trimmed exp + multiplicative causal mask
                            for hi in range(2):
                                sl = slice(hi * QC + col0, (hi + 1) * QC)
                                nc.scalar.activation(al[:, sl], ps[:, sl], EXP,
                                                     scale=0.125)
                                nc.vector.tensor_mul(
                                    al[:, sl], al[:, sl],
                                    cm[:, m * QC + col0:(m + 1) * QC])
                        for hi in range(2):
                            h = 2 * p + hi
                            nc.tensor.matmul(
                                cx[hi][:, col0:QC],
                                vp[:, kb * HG * 65 + h * 65: kb * HG * 65 + (h + 1) * 65],
                                al[:, hi * QC + col0:(hi + 1) * QC],
                                start=(kb == 0), stop=(kb == nkb - 1))
                    # normalize: recip the [1,512] denom rows, broadcast both via
                    # two accumulated outer-product matmuls, then 2 muls into the
                    # pair-stacked ctxu2 (head 2p+hi at partitions hi*64..).
                    rr = nrmp.tile([65, 1024], BF16, tag="rr", name=f"rr{p}_{j}")
                    with nc.allow_low_precision(
                            reason="bf16 softmax denom recip feeds bf16 bcast matmul"):
                        nc.vector.reciprocal(rr[64:65, 0:QC], cx[0][64:65, :])
                        nc.vector.reciprocal(rr[64:65, QC:2 * QC], cx[1][64:65, :])
                    pb = psop.tile([128, 512], F32, tag="pso", name=f"pb{p}_{j}")
                    nc.tensor.matmul(pb[:], ones2[64:65, 0:128],
                                     rr[64:65, 0:QC], start=True, stop=False)
                    nc.tensor.matmul(pb[:], ones2[64:65, 128:256],
                                     rr[64:65, QC:2 * QC], start=False, stop=True)
                    rbs = rbsp.tile([128, 512], F32, tag="rbs", name=f"rbs{p}_{j}")
                    nc.any.tensor_copy(rbs[:], pb[:])
                    for hi in range(2):
                        nc.vector.tensor_mul(
                            ctxu2[hi * 64:(hi + 1) * 64,
                                  p * S + j * QC: p * S + (j + 1) * QC],
                            cx[hi][0:64, :], rbs[hi * 64:(hi + 1) * 64, :])

                # ---- output projection for chunk j: K=128 pair matmuls ----
                cc_in = dramp.tile([2048, QC], BF16, tag=f"cci{j}", name=f"cci{j}")
                for ob in range(8):
                    po = psop.tile([128, 512], F32, tag="pso", name=f"o{j}_{ob}")
                    for pp in range(NP):
                        nc.tensor.matmul(
                            po[:],
                            wo2[:, (pp * 8 + ob) * 128:(pp * 8 + ob + 1) * 128],
                            ctxu2[:, pp * S + j * QC: pp * S + (j + 1) * QC],
                            start=(pp == 0), stop=(pp == NP - 1))
                    # batch-flag staging: own-batch half real, other half zeros
                    stage = stgp.tile([128, 1024], BF16, tag="stage",
                                      name=f"stg{j}_{ob}")
                    nc.vector.tensor_scalar_mul(stage[:, 0:QC], po[:], bsel[:, 0:1])
                    nc.vector.tensor_scalar_mul(stage[:, QC:2 * QC], po[:], bsel[:, 1:2])
                    nc.sync.dma_start(
                        cc_in[ob * 256:(ob + 1) * 256, :]
                          .rearrange("(h p) w -> p h w", h=2),
                        stage[:].rearrange("p (h w) -> p h w", h=2))
                if DEBUG_DUMPS and j == 0:
                    nc.scalar.dma_start(dbg_ccin[:, :], cc_in[:])
                cc_out = dramp.tile([2048, QC], BF16, tag=f"cco{j}", name=f"cco{j}")
                nc.gpsimd.collective_compute(
                    "AllToAll", mybir.AluOpType.bypass,
                    replica_groups=[list(range(NCORE))],
                    ins=[cc_in[:].opt()], outs=[cc_out[:].opt()])
                cc_pairs.append(cc_out)
                if DEBUG_DUMPS and j == 0:
                    nc.scalar.dma_start(dbg_ccout[:, :], cc_out[:])

                # software pipeline: reduce chunk j-1 AFTER chunk j's A2A
                # trigger so the gpsimd queue never delays the next A2A.
                if j > 0:
                    _reduce_chunk(nc, rbp, sump, cc_pairs[j - 1], out_d, j - 1)
            _reduce_chunk(nc, rbp, sump, cc_pairs[NJ - 1], out_d, NJ - 1)
            if DEBUG_DUMPS:
                nc.scalar.dma_start(dbg_qt[:, :], qt[:])
                nc.scalar.dma_start(dbg_kt[:, :], kt[:])
                nc.scalar.dma_start(dbg_ctxu[:, :], ctxu2[:])

    nc.compile()
    return nc


def _reduce_chunk(nc, rbp, sump, cc_out, out_d, j):
    """Read back the A2A result for chunk j, sum the 8 source shards on
    gpsimd, and write this core's owned [128, 2x512] slice of the output."""
    rb = rbp.tile([128, 8 * 2 * QC], BF16, tag="rb", name=f"rb{j}")
    nc.gpsimd.dma_start(
        rb[:].rearrange("p (s h w) -> p s h w", s=8, h=2),
        cc_out[:].rearrange("(s h p) w -> p s h w", p=128, h=2))
    rb4 = rb[:].rearrange("p (s x) -> p s x", s=8)
    acc = sump.tile([128, 2 * QC], F32, tag="acc", name=f"acc{j}")
    nc.gpsimd.tensor_add(acc[:], rb4[:, 0, :], rb4[:, 1, :])
    for s in range(2, 8):
        nc.gpsimd.tensor_add(acc[:], acc[:], rb4[:, s, :])
    nc.gpsimd.dma_start(
        out_d.ap().rearrange("p (h ss) -> p h ss", h=2)[:, :, j * QC:(j + 1) * QC],
        acc[:].rearrange("p (h w) -> p h w", h=2))


_NC = None
_LAST_RESULTS = None  # BassKernelResults of the most recent run (for test harness)


def _causal_mask_tiles() -> np.ndarray:
    r = np.arange(128)[:, None]
    col = np.arange(QC)[None, :]
    tiles = [(col >= r + 128 * m).astype(np.float32) for m in range(4)]
    return np.concatenate(tiles, axis=1)  # [128, 2048]


def _wo2_input(Wo: np.ndarray, g: int) -> np.ndarray:
    """Pair-packed Wo stationary slices: [128, (pp*8 + ob)*128 + c] with
    wo2[r, ...] = Wo[(4g + 2pp + r//64)*64 + r%64, ob*128 + c]."""
    wo2 = np.empty((128, NP * 8 * 128), np.float32)
    for pp in range(NP):
        for half in range(2):
            h = 4 * g + 2 * pp + half
            blk = Wo[h * DH:(h + 1) * DH, :]  # [64, 1024]
            for ob in range(8):
                wo2[half * 64:(half + 1) * 64, (pp * 8 + ob) * 128:
                    (pp * 8 + ob + 1) * 128] = blk[:, ob * 128:(ob + 1) * 128]
    return wo2


def kernel(states, masks, Wq, Wk, Wv, Wo):
    global _NC, _LAST_RESULTS
    if _NC is None:
        _NC = _build()
    bf16 = ml_dtypes.bfloat16
    states = np.asarray(states, np.float32)
    Wq, Wk, Wv, Wo = (np.asarray(w, np.float32) for w in (Wq, Wk, Wv, Wo))
    cm = _causal_mask_tiles().astype(bf16)

    in_maps = []
    for c in range(NCORE):
        b, g = c // 4, c % 4
        cs = slice(g * 256, (g + 1) * 256)
        bsel = np.zeros((128, 2), np.float32)
        bsel[:, b] = 1.0
        in_maps.append({
            "statesT": np.ascontiguousarray(states[b].T).astype(bf16),
            "wq": np.ascontiguousarray(Wq[:, cs]).astype(bf16),
            "wk": np.ascontiguousarray(Wk[:, cs]).astype(bf16),
            "wv": np.ascontiguousarray(Wv[:, cs]).astype(bf16),
            "wo2": _wo2_input(Wo, g).astype(bf16),
            "cmask": cm,
            "bsel": bsel,
        })

    res = bass_utils.run_bass_kernel_spmd(_NC, in_maps, core_ids=list(range(NCORE)))
    _LAST_RESULTS = res
    # core c owns out^T rows [c*128,(c+1)*128) of each batch; stitch + transpose
    out = np.empty((B, S, D), np.float32)
    for c in range(NCORE):
        blk = np.asarray(res.results[c]["out"]).astype(np.float32)  # [128, 2*2048]
        for b in range(B):
            out[b][:, c * 128:(c + 1) * 128] = blk[:, b * S:(b + 1) * S].T
    return out


# revision 9
# speedup vs baseline: 1.4129x; 1.0891x over previous
"""Distributed Bass kernel for causal multi-head attention on 8 TRN2 NeuronCores.

Problem: B=2, S=2048, D=1024, H=16 (dh=64) causal attention layer.
Sharding: core c = (batch b = c//4, head-group g = c%4 covering 4 heads).

Communication (v2): instead of an 8-rank AllReduce of half-zero-padded
partials (2x wire traffic + zero-half matmuls), the output projection
partials are exchanged with a single 8-rank AllToAll per q-chunk and
reduced locally:
  - Output-row ownership: core d owns out^T rows [d*128,(d+1)*128) of BOTH
    batches. Each core's chunk partial [1024 own-batch rows, 512 q] is
    sliced into 8 destination shards of [128, 512].
  - SPMD-uniform batch placement: each A2A shard is [2 batch-halves][128]
    [512]; the staging multiply scales the partial by per-core host flags
    (bsel = [1,0] or [0,1]), so the own-batch half carries the data and the
    other half carries zeros -- no divergent addressing anywhere.
  - After the A2A, 7 elementwise adds (gpsimd, off the DVE critical path)
    sum the 8 source shards; zeros from other-batch sources are harmless.
  - Each core writes only its owned [128, 2*2048] slice; the host stitches
    slices from all 8 cores (no broadcast needed).
  Wire per chunk: 7/8 * 2MB vs AllReduce's ~2 * 2MB, and no zero-half
  output projection matmuls.

Device-side notes:
  - All inputs are pre-cast to bf16 on the host: input DMA volume halves
    and loads run on the HWDGE sync queue (no SWDGE cast).
  - Scores computed transposed [k, q]: softmax-weighted ctx needs no alpha
    transpose; softmax denominator via a ones-column appended to V (M=65).
  - No max-subtraction in softmax (scores ~N(0,1); exp is safe).
  - Causality: k-blocks above the diagonal skipped; diagonal blocks are
    column-trimmed (matmul/exp/mask/ctx restricted to cols >= m*128) and
    get a multiplicative 0/1 mask.
  - Normalization: reciprocal on the [1,512] denominator rows, one psum
    [128,512] broadcast via two accumulated outer-product matmuls, one
    psum->sbuf copy, two muls into the pair-stacked ctxu2.
  - ctxu2 stacks head pairs on partitions (head 2p+hi at partitions
    [hi*64,(hi+1)*64)), so output projection contracts K=128 (2 heads per
    matmul) -- half the matmuls of the K=64 variant.
"""

import numpy as np
import ml_dtypes

import concourse.bass as bass
import concourse.bacc as bacc
import concourse.mybir as mybir
import concourse.tile as tile
from concourse import bass_utils

DEBUG_DUMPS = False

F32 = mybir.dt.float32
BF16 = mybir.dt.bfloat16
EXP = mybir.ActivationFunctionType.Exp

B, S, D, H = 2, 2048, 1024, 16
DH = 64            # head dim
HG = 4             # heads per core (head group)
NP = 2             # head pairs per core
QC = 512           # q-chunk
NJ = S // QC       # 4 q-chunks
KB = 128           # k block (partition tile)
NKB = S // KB      # 16 k blocks
DB = D // 128      # 8 contraction blocks of 128
NCORE = 8


def _build():
    nc = bacc.Bacc(
        "TRN2", target_bir_lowering=False, debug=False,
        enable_asserts=False, num_devices=NCORE,
    )

    statesT = nc.dram_tensor("statesT", [D, S], BF16, kind="ExternalInput")
    wq_d = nc.dram_tensor("wq", [D, 256], BF16, kind="ExternalInput")
    wk_d = nc.dram_tensor("wk", [D, 256], BF16, kind="ExternalInput")
    wv_d = nc.dram_tensor("wv", [D, 256], BF16, kind="ExternalInput")
    # wo2[r, (pp*8 + ob)*128 + c] = Wo[(4g + 2pp + r//64)*64 + r%64, ob*128 + c]
    wo2_d = nc.dram_tensor("wo2", [128, NP * 8 * 128], BF16, kind="ExternalInput")
    cm_d = nc.dram_tensor("cmask", [128, 4 * QC], BF16, kind="ExternalInput")
    bsel_d = nc.dram_tensor("bsel", [128, 2], F32, kind="ExternalInput")
    # out: core-owned slice; [r, b*2048 + s] = out^T[own_row_base + r, s] for batch b
    out_d = nc.dram_tensor("out", [128, B * S], BF16, kind="ExternalOutput")
    if DEBUG_DUMPS:
        dbg_qt = nc.dram_tensor("dbg_qt", [128, NP * S], BF16, kind="ExternalOutput")
        dbg_kt = nc.dram_tensor("dbg_kt", [128, NP * S], BF16, kind="ExternalOutput")
        dbg_ctxu = nc.dram_tensor("dbg_ctxu", [128, NP * S], BF16, kind="ExternalOutput")
        dbg_ccin = nc.dram_tensor("dbg_ccin", [2048, QC], BF16, kind="ExternalOutput")
        dbg_ccout = nc.dram_tensor("dbg_ccout", [2048, QC], BF16, kind="ExternalOutput")

    with tile.TileContext(nc) as tc:
        with (
            tc.tile_pool(name="const", bufs=1) as constp,
            tc.tile_pool(name="alpha", bufs=3) as alphap,
            tc.tile_pool(name="nrm", bufs=2) as nrmp,
            tc.tile_pool(name="rbs", bufs=2) as rbsp,
            tc.tile_pool(name="stg", bufs=4) as stgp,
            tc.tile_pool(name="rb", bufs=2) as rbp,
            tc.tile_pool(name="sum", bufs=2) as sump,
            tc.tile_pool(name="ps", bufs=2, space="PSUM") as psp,
            tc.tile_pool(name="psc", bufs=2, space="PSUM") as pscp,
            tc.tile_pool(name="pso", bufs=2, space="PSUM") as psop,
            tc.tile_pool(name="dram", bufs=1, space="DRAM") as dramp,
        ):
            # ---------------- persistent SBUF tensors ----------------
            st = constp.tile([128, DB * S], BF16, tag="st")        # statesT: [d%128, db*S + s]
            wq = constp.tile([128, DB * 256], BF16, tag="wq")      # [d%128, db*256 + c]
            wk = constp.tile([128, DB * 256], BF16, tag="wk")
            wv = constp.tile([128, DB * 256], BF16, tag="wv")
            wo2 = constp.tile([128, NP * 8 * 128], BF16, tag="wo2")
            cm = constp.tile([128, 4 * QC], BF16, tag="cm")        # 4 causal mask alignments
            bsel = constp.tile([128, 2], F32, tag="bsel")
            qt = constp.tile([128, NP * S], BF16, tag="qt")        # [pair-local c, p*S + q]
            kt = constp.tile([128, NP * S], BF16, tag="kt")
            vp = constp.tile([128, NKB * (HG * 65)], BF16, tag="vp")  # V' 65-pitch + ones col
            ctxu2 = constp.tile([128, NP * S], BF16, tag="ctxu2")  # pair-stacked normalized ctx^T
            ones2 = constp.tile([65, 256], BF16, tag="ones2")       # bcast outer-product lhsTs

            # -------- loads (all bf16; HWDGE sync queue) --------
            st_view = statesT.ap().rearrange("(a p) s -> p a s", p=128)
            nc.sync.dma_start(
                wq[:].rearrange("p (a c) -> p a c", a=DB),
                wq_d.ap().rearrange("(a p) c -> p a c", p=128))
            for db in range(DB):  # chunked so QKV matmuls can start early
                nc.sync.dma_start(st[:, db * S:(db + 1) * S], st_view[:, db, :])
            nc.sync.dma_start(
                wk[:].rearrange("p (a c) -> p a c", a=DB),
                wk_d.ap().rearrange("(a p) c -> p a c", p=128))
            nc.sync.dma_start(
                wv[:].rearrange("p (a c) -> p a c", a=DB),
                wv_d.ap().rearrange("(a p) c -> p a c", p=128))
            nc.sync.dma_start(cm[:], cm_d[:, :])
            nc.sync.dma_start(wo2[:], wo2_d[:, :])
            nc.sync.dma_start(bsel[:], bsel_d[:, :])

            nc.vector.memset(ones2[64:65, :], 0.0)
            nc.vector.memset(ones2[64:65, 0:64], 1.0)     # h0: out partitions 0-63
            nc.vector.memset(ones2[64:65, 192:256], 1.0)  # h1: out partitions 64-127
            # V' ones columns (denominator trick)
            nc.vector.memset(
                vp[:].rearrange("p (n w) -> p n w", w=65)[:, :, 64:65], 1.0)

            # ---------------- QKV projection emitters ----------------
            def emit_qk(dst, w_sb, p, jj):
                ps = psp.tile([128, 1024], F32, tag="ps",
                              name=f"qk{p}_{jj}_{dst is kt}")
                for half in range(2):
                    q0 = jj * 1024 + half * QC
                    for db in range(DB):
                        nc.tensor.matmul(
                            ps[:, half * QC:(half + 1) * QC],
                            w_sb[:, db * 256 + p * 128: db * 256 + (p + 1) * 128],
                            st[:, db * S + q0: db * S + q0 + QC],
                            start=(db == 0), stop=(db == DB - 1))
                nc.any.tensor_copy(
                    dst[:, p * S + jj * 1024: p * S + (jj + 1) * 1024], ps[:])

            def emit_v(kb):
                # V in [s, c] layout, written into 65-pitch V' slots
                psv = psop.tile([128, 512], F32, tag="pso", name=f"v{kb}")
                for db in range(DB):
                    nc.tensor.matmul(
                        psv[:, 0:256],
                        st[:, db * S + kb * KB: db * S + (kb + 1) * KB],
                        wv[:, db * 256:(db + 1) * 256],
                        start=(db == 0), stop=(db == DB - 1))
                nc.any.tensor_copy(
                    vp[:, kb * HG * 65:(kb + 1) * HG * 65]
                      .rearrange("p (h w) -> p h w", w=65)[:, :, 0:64],
                    psv[:, 0:256].rearrange("p (h w) -> p h w", w=64))

            # interleave: emit only what chunk j needs before its attention,
            # so the first A2A fires as early as possible.
            qkv_stages = {
                0: lambda: ([emit_qk(kt, wk, p, 0) for p in range(NP)],
                            [emit_qk(qt, wq, p, 0) for p in range(NP)],
                            [emit_v(kb) for kb in range(0, 4)]),
                1: lambda: ([emit_qk(kt, wk, p, 1) for p in range(NP)],
                            [emit_qk(qt, wq, p, 1) for p in range(NP)],
                            [emit_v(kb) for kb in range(4, 8)]),
                2: lambda: [emit_v(kb) for kb in range(8, 16)],
            }

            # ---- attention (j-outer; A2A per q-chunk overlaps compute) ----
            cc_pairs = []
            for j in range(NJ):
                if j in qkv_stages:
                    qkv_stages[j]()
                for p in range(NP):
                    cx = [pscp.tile([65, QC], F32, tag="psc",
                                    name=f"cx{p}_{j}_{hi}") for hi in range(2)]
                    nkb = 4 * j + 4
                    for kb in range(nkb):
                        m = kb - 4 * j
                        col0 = max(m, 0) * 128  # diagonal column trim
                        ps = psp.tile([128, 1024], F32, tag="ps",
                                      name=f"s{p}_{j}_{kb}")
                        for hi in range(2):
                            h0 = hi * 64
                            nc.tensor.matmul(
                                ps[:, hi * QC + col0:(hi + 1) * QC],
                                kt[h0:h0 + 64, p * S + kb * KB: p * S + (kb + 1) * KB],
                                qt[h0:h0 + 64, p * S + j * QC + col0: p * S + (j + 1) * QC],
                                start=True, stop=True)
                        al = alphap.tile([128, 1024], BF16, tag="alpha",
                                         name=f"al{p}_{j}_{kb}")
                        if m < 0:
                            nc.scalar.activation(al[:], ps[:], EXP, scale=0.125)
                        else:  # diagonal: trimmed exp + multiplicative causal mask
                            for hi in range(2):
                                sl = slice(hi * QC + col0, (hi + 1) * QC)
                                nc.scalar.activation(al[:, sl], ps[:, sl], EXP,
                                                     scale=0.125)
                                nc.vector.tensor_mul(
                                    al[:, sl], al[:, sl],
                                    cm[:, m * QC + col0:(m + 1) * QC])
                        for hi in range(2):
                            h = 2 * p + hi
                            nc.tensor.matmul(
                                cx[hi][:, col0:QC],
                                vp[:, kb * HG * 65 + h * 65: kb * HG * 65 + (h + 1) * 65],
                                al[:, hi * QC + col0:(hi + 1) * QC],
                                start=(kb == 0), stop=(kb == nkb - 1))
                    # normalize: recip the [1,512] denom rows, broadcast both via
                    # two accumulated outer-product matmuls, then 2 muls into the
                    # pair-stacked ctxu2 (head 2p+hi at partitions hi*64..).
                    rr = nrmp.tile([65, 1024], BF16, tag="rr", name=f"rr{p}_{j}")
                    with nc.allow_low_precision(
                            reason="bf16 softmax denom recip feeds bf16 bcast matmul"):
                        nc.vector.reciprocal(rr[64:65, 0:QC], cx[0][64:65, :])
                        nc.vector.reciprocal(rr[64:65, QC:2 * QC], cx[1][64:65, :])
                    pb = psop.tile([128, 512], F32, tag="pso", name=f"pb{p}_{j}")
                    nc.tensor.matmul(pb[:], ones2[64:65, 0:128],
                                     rr[64:65, 0:QC], start=True, stop=False)
                    nc.tensor.matmul(pb[:], ones2[64:65, 128:256],
                                     rr[64:65, QC:2 * QC], start=False, stop=True)
                    rbs = rbsp.tile([128, 512], F32, tag="rbs", name=f"rbs{p}_{j}")
                    nc.any.tensor_copy(rbs[:], pb[:])
                    for hi in range(2):
                        nc.vector.tensor_mul(
                            ctxu2[hi * 64:(hi + 1) * 64,
                                  p * S + j * QC: p * S + (j + 1) * QC],
                            cx[hi][0:64, :], rbs[hi * 64:(hi + 1) * 64, :])

                # ---- output projection for chunk j: K=128 pair matmuls ----
                cc_in = dramp.tile([2048, QC], BF16, tag=f"cci{j}", name=f"cci{j}")
                for ob in range(8):
                    po = psop.tile([128, 512], F32, tag="pso", name=f"o{j}_{ob}")
                    for pp in range(NP):
                        nc.tensor.matmul(
                            po[:],
                            wo2[:, (pp * 8 + ob) * 128:(pp * 8 + ob + 1) * 128],
                            ctxu2[:, pp * S + j * QC: pp * S + (j + 1) * QC],
                            start=(pp == 0), stop=(pp == NP - 1))
                    # batch-flag staging: own-batch half real, other half zeros
                    stage = stgp.tile([128, 1024], BF16, tag="stage",
                                      name=f"stg{j}_{ob}")
                    nc.scalar.activation(stage[:, 0:QC], po[:],
                                         mybir.ActivationFunctionType.Copy,
                                         scale=bsel[:, 0:1])
                    nc.vector.tensor_scalar_mul(stage[:, QC:2 * QC], po[:], bsel[:, 1:2])
                    nc.sync.dma_start(
                        cc_in[ob * 256:(ob + 1) * 256, :]
                          .rearrange("(h p) w -> p h w", h=2),
                        stage[:].rearrange("p (h w) -> p h w", h=2))
                if DEBUG_DUMPS and j == 0:
                    nc.scalar.dma_start(dbg_ccin[:, :], cc_in[:])
                cc_out = dramp.tile([2048, QC], BF16, tag=f"cco{j}", name=f"cco{j}")
                nc.gpsimd.collective_compute(
                    "AllToAll", mybir.AluOpType.bypass,
                    replica_groups=[list(range(NCORE))],
                    ins=[cc_in[:].opt()], outs=[cc_out[:].opt()])
                cc_pairs.append(cc_out)
                if DEBUG_DUMPS and j == 0:
                    nc.scalar.dma_start(dbg_ccout[:, :], cc_out[:])

                # software pipeline (lag 2): reduce chunk j-2 after chunk j's
                # A2A trigger so neither the gpsimd trigger queue nor the
                # vector queue ever stalls waiting on an in-flight A2A.
                if j >= 2:
                    _reduce_chunk(nc, rbp, sump, cc_pairs[j - 2], out_d, j - 2)
            _reduce_chunk(nc, rbp, sump, cc_pairs[NJ - 2], out_d, NJ - 2)
            _reduce_chunk(nc, rbp, sump, cc_pairs[NJ - 1], out_d, NJ - 1)
            if DEBUG_DUMPS:
                nc.scalar.dma_start(dbg_qt[:, :], qt[:])
                nc.scalar.dma_start(dbg_kt[:, :], kt[:])
                nc.scalar.dma_start(dbg_ctxu[:, :], ctxu2[:])

    nc.compile()
    return nc


def _reduce_chunk(nc, rbp, sump, cc_out, out_d, j):
    """Read back the A2A result for chunk j (sync HWDGE), tree-sum the 8
    source shards on vector in bf16, and write this core's owned
    [128, 2x512] slice of the output (sync, no cast)."""
    rb = rbp.tile([128, 8 * 2 * QC], BF16, tag="rb", name=f"rb{j}")
    nc.sync.dma_start(
        rb[:].rearrange("p (s h w) -> p s h w", s=8, h=2),
        cc_out[:].rearrange("(s h p) w -> p s h w", p=128, h=2))
    rb4 = rb[:].rearrange("p (s x) -> p s x", s=8)
    t0 = sump.tile([128, 2 * QC], BF16, tag="t0", name=f"t0_{j}")
    t1 = sump.tile([128, 2 * QC], BF16, tag="t1", name=f"t1_{j}")
    t2 = sump.tile([128, 2 * QC], BF16, tag="t2", name=f"t2_{j}")
    with nc.allow_low_precision(reason="bf16 tree-sum of 8 A2A shards"):
        nc.vector.tensor_add(t0[:], rb4[:, 0, :], rb4[:, 1, :])
        nc.vector.tensor_add(t1[:], rb4[:, 2, :], rb4[:, 3, :])
        nc.vector.tensor_add(t0[:], t0[:], t1[:])
        nc.vector.tensor_add(t1[:], rb4[:, 4, :], rb4[:, 5, :])
        nc.vector.tensor_add(t2[:], rb4[:, 6, :], rb4[:, 7, :])
        nc.vector.tensor_add(t1[:], t1[:], t2[:])
        nc.vector.tensor_add(t0[:], t0[:], t1[:])
    nc.sync.dma_start(
        out_d.ap().rearrange("p (h ss) -> p h ss", h=2)[:, :, j * QC:(j + 1) * QC],
        t0[:].rearrange("p (h w) -> p h w", h=2))


_NC = None
_LAST_RESULTS = None  # BassKernelResults of the most recent run (for test harness)


def _causal_mask_tiles() -> np.ndarray:
    r = np.arange(128)[:, None]
    col = np.arange(QC)[None, :]
    tiles = [(col >= r + 128 * m).astype(np.float32) for m in range(4)]
    return np.concatenate(tiles, axis=1)  # [128, 2048]


def _wo2_input(Wo: np.ndarray, g: int) -> np.ndarray:
    """Pair-packed Wo stationary slices: [128, (pp*8 + ob)*128 + c] with
    wo2[r, ...] = Wo[(4g + 2pp + r//64)*64 + r%64, ob*128 + c]."""
    wo2 = np.empty((128, NP * 8 * 128), np.float32)
    for pp in range(NP):
        for half in range(2):
            h = 4 * g + 2 * pp + half
            blk = Wo[h * DH:(h + 1) * DH, :]  # [64, 1024]
            for ob in range(8):
                wo2[half * 64:(half + 1) * 64, (pp * 8 + ob) * 128:
                    (pp * 8 + ob + 1) * 128] = blk[:, ob * 128:(ob + 1) * 128]
    return wo2


def kernel(states, masks, Wq, Wk, Wv, Wo):
    global _NC, _LAST_RESULTS
    if _NC is None:
        _NC = _build()
    bf16 = ml_dtypes.bfloat16
    states = np.asarray(states, np.float32)
    Wq, Wk, Wv, Wo = (np.asarray(w, np.float32) for w in (Wq, Wk, Wv, Wo))
    cm = _causal_mask_tiles().astype(bf16)

    in_maps = []
    for c in range(NCORE):
        b, g = c // 4, c % 4
        cs = slice(g * 256, (g + 1) * 256)
        bsel = np.zeros((128, 2), np.float32)
        bsel[:, b] = 1.0
        in_maps.append({
            "statesT": np.ascontiguousarray(states[b].T).astype(bf16),
            "wq": np.ascontiguousarray(Wq[:, cs]).astype(bf16),
            "wk": np.ascontiguousarray(Wk[:, cs]).astype(bf16),
            "wv": np.ascontiguousarray(Wv[:, cs]).astype(bf16),
            "wo2": _wo2_input(Wo, g).astype(bf16),
            "cmask": cm,
            "bsel": bsel,
        })

    res = bass_utils.run_bass_kernel_spmd(_NC, in_maps, core_ids=list(range(NCORE)))
    _LAST_RESULTS = res
    # core c owns out^T rows [c*128,(c+1)*128) of each batch; stitch + transpose
    out = np.empty((B, S, D), np.float32)
    for c in range(NCORE):
        blk = np.asarray(res.results[c]["out"]).astype(np.float32)  # [128, 2*2048]
        for b in range(B):
            out[b][:, c * 128:(c + 1) * 128] = blk[:, b * S:(b + 1) * S].T
    return out


# revision 11
# speedup vs baseline: 1.4139x; 1.0008x over previous
"""Distributed Bass kernel for causal multi-head attention on 8 TRN2 NeuronCores.

Problem: B=2, S=2048, D=1024, H=16 (dh=64) causal attention layer.
Sharding: core c = (batch b = c//4, head-group g = c%4 covering 4 heads).

Communication (v2): instead of an 8-rank AllReduce of half-zero-padded
partials (2x wire traffic + zero-half matmuls), the output projection
partials are exchanged with a single 8-rank AllToAll per q-chunk and
reduced locally:
  - Output-row ownership: core d owns out^T rows [d*128,(d+1)*128) of BOTH
    batches. Each core's chunk partial [1024 own-batch rows, 512 q] is
    sliced into 8 destination shards of [128, 512].
  - SPMD-uniform batch placement: each A2A shard is [2 batch-halves][128]
    [512]; the staging multiply scales the partial by per-core host flags
    (bsel = [1,0] or [0,1]), so the own-batch half carries the data and the
    other half carries zeros -- no divergent addressing anywhere.
  - After the A2A, 7 elementwise adds (gpsimd, off the DVE critical path)
    sum the 8 source shards; zeros from other-batch sources are harmless.
  - Each core writes only its owned [128, 2*2048] slice; the host stitches
    slices from all 8 cores (no broadcast needed).
  Wire per chunk: 7/8 * 2MB vs AllReduce's ~2 * 2MB, and no zero-half
  output projection matmuls.

Device-side notes:
  - All inputs are pre-cast to bf16 on the host: input DMA volume halves
    and loads run on the HWDGE sync queue (no SWDGE cast).
  - Scores computed transposed [k, q]: softmax-weighted ctx needs no alpha
    transpose; softmax denominator via a ones-column appended to V (M=65).
  - No max-subtraction in softmax (scores ~N(0,1); exp is safe).
  - Causality: k-blocks above the diagonal skipped; diagonal blocks are
    column-trimmed (matmul/exp/mask/ctx restricted to cols >= m*128) and
    get a multiplicative 0/1 mask.
  - Normalization: reciprocal on the [1,512] denominator rows, one psum
    [128,512] broadcast via two accumulated outer-product matmuls, one
    psum->sbuf copy, two muls into the pair-stacked ctxu2.
  - ctxu2 stacks head pairs on partitions (head 2p+hi at partitions
    [hi*64,(hi+1)*64)), so output projection contracts K=128 (2 heads per
    matmul) -- half the matmuls of the K=64 variant.
"""

import numpy as np
import ml_dtypes

import concourse.bass as bass
import concourse.bacc as bacc
import concourse.mybir as mybir
import concourse.tile as tile
from concourse import bass_utils

DEBUG_DUMPS = False

F32 = mybir.dt.float32
BF16 = mybir.dt.bfloat16
EXP = mybir.ActivationFunctionType.Exp

B, S, D, H = 2, 2048, 1024, 16
DH = 64            # head dim
HG = 4             # heads per core (head group)
NP = 2             # head pairs per core
QC = 512           # q-chunk
NJ = S // QC       # 4 q-chunks
KB = 128           # k block (partition tile)
NKB = S // KB      # 16 k blocks
DB = D // 128      # 8 contraction blocks of 128
NCORE = 8


def _build():
    nc = bacc.Bacc(
        "TRN2", target_bir_lowering=False, debug=False,
        enable_asserts=False, num_devices=NCORE,
    )

    statesT = nc.dram_tensor("statesT", [D, S], BF16, kind="ExternalInput")
    wq_d = nc.dram_tensor("wq", [D, 256], BF16, kind="ExternalInput")
    wk_d = nc.dram_tensor("wk", [D, 256], BF16, kind="ExternalInput")
    wv_d = nc.dram_tensor("wv", [D, 256], BF16, kind="ExternalInput")
    # wo2[r, (pp*8 + ob)*128 + c] = Wo[(4g + 2pp + r//64)*64 + r%64, ob*128 + c]
    wo2_d = nc.dram_tensor("wo2", [128, NP * 8 * 128], BF16, kind="ExternalInput")
    cm_d = nc.dram_tensor("cmask", [128, 4 * QC], BF16, kind="ExternalInput")
    bsel_d = nc.dram_tensor("bsel", [128, 2], F32, kind="ExternalInput")
    # out: core-owned slice; [r, b*2048 + s] = out^T[own_row_base + r, s] for batch b
    out_d = nc.dram_tensor("out", [128, B * S], BF16, kind="ExternalOutput")
    if DEBUG_DUMPS:
        dbg_qt = nc.dram_tensor("dbg_qt", [128, NP * S], BF16, kind="ExternalOutput")
        dbg_kt = nc.dram_tensor("dbg_kt", [128, NP * S], BF16, kind="ExternalOutput")
        dbg_ctxu = nc.dram_tensor("dbg_ctxu", [128, NP * S], BF16, kind="ExternalOutput")
        dbg_ccin = nc.dram_tensor("dbg_ccin", [2048, QC], BF16, kind="ExternalOutput")
        dbg_ccout = nc.dram_tensor("dbg_ccout", [2048, QC], BF16, kind="ExternalOutput")

    with tile.TileContext(nc) as tc:
        with (
            tc.tile_pool(name="const", bufs=1) as constp,
            tc.tile_pool(name="alpha", bufs=3) as alphap,
            tc.tile_pool(name="nrm", bufs=2) as nrmp,
            tc.tile_pool(name="rbs", bufs=2) as rbsp,
            tc.tile_pool(name="stg", bufs=4) as stgp,
            tc.tile_pool(name="rb", bufs=2) as rbp,
            tc.tile_pool(name="sum", bufs=2) as sump,
            tc.tile_pool(name="ps", bufs=2, space="PSUM") as psp,
            tc.tile_pool(name="psc", bufs=2, space="PSUM") as pscp,
            tc.tile_pool(name="pso", bufs=2, space="PSUM") as psop,
            tc.tile_pool(name="dram", bufs=1, space="DRAM") as dramp,
        ):
            # ---------------- persistent SBUF tensors ----------------
            st = constp.tile([128, DB * S], BF16, tag="st")        # statesT: [d%128, db*S + s]
            wq = constp.tile([128, DB * 256], BF16, tag="wq")      # [d%128, db*256 + c]
            wk = constp.tile([128, DB * 256], BF16, tag="wk")
            wv = constp.tile([128, DB * 256], BF16, tag="wv")
            wo2 = constp.tile([128, NP * 8 * 128], BF16, tag="wo2")
            cm = constp.tile([128, 4 * QC], BF16, tag="cm")        # 4 causal mask alignments
            bsel = constp.tile([128, 2], F32, tag="bsel")
            qt = constp.tile([128, NP * S], BF16, tag="qt")        # [pair-local c, p*S + q]
            kt = constp.tile([128, NP * S], BF16, tag="kt")
            vp = constp.tile([128, NKB * (HG * 65)], BF16, tag="vp")  # V' 65-pitch + ones col
            ctxu2 = constp.tile([128, NP * S], BF16, tag="ctxu2")  # pair-stacked normalized ctx^T
            ones2 = constp.tile([65, 256], BF16, tag="ones2")       # bcast outer-product lhsTs

            # -------- loads (all bf16; split across sync+scalar HWDGE) --------
            st_view = statesT.ap().rearrange("(a p) s -> p a s", p=128)
            nc.sync.dma_start(
                wk[:].rearrange("p (a c) -> p a c", a=DB),
                wk_d.ap().rearrange("(a p) c -> p a c", p=128))
            nc.sync.dma_start(
                wq[:].rearrange("p (a c) -> p a c", a=DB),
                wq_d.ap().rearrange("(a p) c -> p a c", p=128))
            for db in range(DB):  # chunked + 2 queues so QKV can start early
                eng = nc.sync if db % 2 == 0 else nc.scalar
                eng.dma_start(st[:, db * S:(db + 1) * S], st_view[:, db, :])
            nc.scalar.dma_start(
                wv[:].rearrange("p (a c) -> p a c", a=DB),
                wv_d.ap().rearrange("(a p) c -> p a c", p=128))
            nc.scalar.dma_start(cm[:], cm_d[:, :])
            nc.scalar.dma_start(wo2[:], wo2_d[:, :])
            nc.scalar.dma_start(bsel[:], bsel_d[:, :])

            nc.vector.memset(ones2[64:65, :], 0.0)
            nc.vector.memset(ones2[64:65, 0:64], 1.0)     # h0: out partitions 0-63
            nc.vector.memset(ones2[64:65, 192:256], 1.0)  # h1: out partitions 64-127
            # V' ones columns (denominator trick)
            nc.vector.memset(
                vp[:].rearrange("p (n w) -> p n w", w=65)[:, :, 64:65], 1.0)

            # ---------------- QKV projection emitters ----------------
            def emit_qk(dst, w_sb, p, jj):
                ps = psp.tile([128, 1024], F32, tag="ps",
                              name=f"qk{p}_{jj}_{dst is kt}")
                for half in range(2):
                    q0 = jj * 1024 + half * QC
                    for db in range(DB):
                        nc.tensor.matmul(
                            ps[:, half * QC:(half + 1) * QC],
                            w_sb[:, db * 256 + p * 128: db * 256 + (p + 1) * 128],
                            st[:, db * S + q0: db * S + q0 + QC],
                            start=(db == 0), stop=(db == DB - 1))
                nc.any.tensor_copy(
                    dst[:, p * S + jj * 1024: p * S + (jj + 1) * 1024], ps[:])

            def emit_v(kb):
                # V in [s, c] layout, written into 65-pitch V' slots
                psv = psop.tile([128, 512], F32, tag="pso", name=f"v{kb}")
                for db in range(DB):
                    nc.tensor.matmul(
                        psv[:, 0:256],
                        st[:, db * S + kb * KB: db * S + (kb + 1) * KB],
                        wv[:, db * 256:(db + 1) * 256],
                        start=(db == 0), stop=(db == DB - 1))
                nc.any.tensor_copy(
                    vp[:, kb * HG * 65:(kb + 1) * HG * 65]
                      .rearrange("p (h w) -> p h w", w=65)[:, :, 0:64],
                    psv[:, 0:256].rearrange("p (h w) -> p h w", w=64))

            # interleave: emit only what chunk j needs before its attention,
            # so the first A2A fires as early as possible.
            qkv_stages = {
                0: lambda: ([emit_qk(kt, wk, p, 0) for p in range(NP)],
                            [emit_qk(qt, wq, p, 0) for p in range(NP)],
                            [emit_v(kb) for kb in range(0, 4)]),
                1: lambda: ([emit_qk(kt, wk, p, 1) for p in range(NP)],
                            [emit_qk(qt, wq, p, 1) for p in range(NP)],
                            [emit_v(kb) for kb in range(4, 8)]),
                2: lambda: [emit_v(kb) for kb in range(8, 12)],
                3: lambda: [emit_v(kb) for kb in range(12, 16)],
            }

            # ---- attention (j-outer; A2A per q-chunk overlaps compute) ----
            def emit_attn_pair(j, p):
                """Scores+ctx (kb loop) for pair p of chunk j; returns cx."""
                cx = [pscp.tile([65, QC], F32, tag="psc",
                                name=f"cx{p}_{j}_{hi}") for hi in range(2)]
                nkb = 4 * j + 4
                for kb in range(nkb):
                    m = kb - 4 * j
                    col0 = max(m, 0) * 128  # diagonal column trim
                    ps = psp.tile([128, 1024], F32, tag="ps",
                                  name=f"s{p}_{j}_{kb}")
                    for hi in range(2):
                        h0 = hi * 64
                        nc.tensor.matmul(
                            ps[:, hi * QC + col0:(hi + 1) * QC],
                            kt[h0:h0 + 64, p * S + kb * KB: p * S + (kb + 1) * KB],
                            qt[h0:h0 + 64, p * S + j * QC + col0: p * S + (j + 1) * QC],
                            start=True, stop=True)
                    al = alphap.tile([128, 1024], BF16, tag="alpha",
                                     name=f"al{p}_{j}_{kb}")
                    if m < 0:
                        nc.scalar.activation(al[:], ps[:], EXP, scale=0.125)
                    else:  # diagonal: trimmed exp + multiplicative causal mask
                        for hi in range(2):
                            sl = slice(hi * QC + col0, (hi + 1) * QC)
                            nc.scalar.activation(al[:, sl], ps[:, sl], EXP,
                                                 scale=0.125)
                            nc.vector.tensor_mul(
                                al[:, sl], al[:, sl],
                                cm[:, m * QC + col0:(m + 1) * QC])
                    for hi in range(2):
                        h = 2 * p + hi
                        nc.tensor.matmul(
                            cx[hi][:, col0:QC],
                            vp[:, kb * HG * 65 + h * 65: kb * HG * 65 + (h + 1) * 65],
                            al[:, hi * QC + col0:(hi + 1) * QC],
                            start=(kb == 0), stop=(kb == nkb - 1))
                return cx

            def emit_norm(j, p, cx):
                # normalize: recip the [1,512] denom rows, broadcast both via
                # two accumulated outer-product matmuls, then 2 muls into the
                # pair-stacked ctxu2 (head 2p+hi at partitions hi*64..).
                rr = nrmp.tile([65, 1024], BF16, tag="rr", name=f"rr{p}_{j}")
                with nc.allow_low_precision(
                        reason="bf16 softmax denom recip feeds bf16 bcast matmul"):
                    nc.vector.reciprocal(rr[64:65, 0:QC], cx[0][64:65, :])
                    nc.vector.reciprocal(rr[64:65, QC:2 * QC], cx[1][64:65, :])
                pb = psop.tile([128, 512], F32, tag="pso", name=f"pb{p}_{j}")
                nc.tensor.matmul(pb[:], ones2[64:65, 0:128],
                                 rr[64:65, 0:QC], start=True, stop=False)
                nc.tensor.matmul(pb[:], ones2[64:65, 128:256],
                                 rr[64:65, QC:2 * QC], start=False, stop=True)
                rbs = rbsp.tile([128, 512], F32, tag="rbs", name=f"rbs{p}_{j}")
                nc.any.tensor_copy(rbs[:], pb[:])
                for hi in range(2):
                    nc.vector.tensor_mul(
                        ctxu2[hi * 64:(hi + 1) * 64,
                              p * S + j * QC: p * S + (j + 1) * QC],
                        cx[hi][0:64, :], rbs[hi * 64:(hi + 1) * 64, :])

            cc_pairs = []

            def emit_outproj(j):
                # ---- output projection for chunk j: K=128 pair matmuls ----
                cc_in = dramp.tile([2048, QC], BF16, tag=f"cci{j}", name=f"cci{j}")
                for ob in range(8):
                    po = psop.tile([128, 512], F32, tag="pso", name=f"o{j}_{ob}")
                    for pp in range(NP):
                        nc.tensor.matmul(
                            po[:],
                            wo2[:, (pp * 8 + ob) * 128:(pp * 8 + ob + 1) * 128],
                            ctxu2[:, pp * S + j * QC: pp * S + (j + 1) * QC],
                            start=(pp == 0), stop=(pp == NP - 1))
                    # batch-flag staging: own-batch half real, other half zeros
                    stage = stgp.tile([128, 1024], BF16, tag="stage",
                                      name=f"stg{j}_{ob}")
                    nc.scalar.activation(stage[:, 0:QC], po[:],
                                         mybir.ActivationFunctionType.Copy,
                                         scale=bsel[:, 0:1])
                    nc.vector.tensor_scalar_mul(stage[:, QC:2 * QC], po[:], bsel[:, 1:2])
                    nc.sync.dma_start(
                        cc_in[ob * 256:(ob + 1) * 256, :]
                          .rearrange("(h p) w -> p h w", h=2),
                        stage[:].rearrange("p (h w) -> p h w", h=2))
                if DEBUG_DUMPS and j == 0:
                    nc.scalar.dma_start(dbg_ccin[:, :], cc_in[:])
                cc_out = dramp.tile([2048, QC], BF16, tag=f"cco{j}", name=f"cco{j}")
                nc.gpsimd.collective_compute(
                    "AllToAll", mybir.AluOpType.bypass,
                    replica_groups=[list(range(NCORE))],
                    ins=[cc_in[:].opt()], outs=[cc_out[:].opt()])
                cc_pairs.append(cc_out)
                if DEBUG_DUMPS and j == 0:
                    nc.scalar.dma_start(dbg_ccout[:, :], cc_out[:])

            # Chunk j's output projection is emitted in the middle of chunk
            # j+1's attention: the next chunk's scores hide the norm-chain
            # latency, and the projection reads ctxu2 written one chunk ago.
            for j in range(NJ):
                if j in qkv_stages:
                    qkv_stages[j]()
                for p in range(NP):
                    cx = emit_attn_pair(j, p)
                    if p == 0 and j > 0:
                        emit_outproj(j - 1)
                    emit_norm(j, p, cx)
            emit_outproj(NJ - 1)
            # All reductions after the loop: A2As for chunks 0..2 completed
            # long ago; only chunk 3's is waited on (the tail).
            for j in range(NJ):
                _reduce_chunk(nc, rbp, sump, cc_pairs[j], out_d, j)
            if DEBUG_DUMPS:
                nc.scalar.dma_start(dbg_qt[:, :], qt[:])
                nc.scalar.dma_start(dbg_kt[:, :], kt[:])
                nc.scalar.dma_start(dbg_ctxu[:, :], ctxu2[:])

    nc.compile()
    return nc


def _reduce_chunk(nc, rbp, sump, cc_out, out_d, j):
    """Read back the A2A result for chunk j (sync HWDGE), tree-sum the 8
    source shards on vector in bf16, and write this core's owned
    [128, 2x512] slice of the output (sync, no cast)."""
    rb = rbp.tile([128, 8 * 2 * QC], BF16, tag="rb", name=f"rb{j}")
    nc.sync.dma_start(
        rb[:].rearrange("p (s h w) -> p s h w", s=8, h=2),
        cc_out[:].rearrange("(s h p) w -> p s h w", p=128, h=2))
    rb4 = rb[:].rearrange("p (s x) -> p s x", s=8)
    t0 = sump.tile([128, 2 * QC], BF16, tag="t0", name=f"t0_{j}")
    t1 = sump.tile([128, 2 * QC], BF16, tag="t1", name=f"t1_{j}")
    t2 = sump.tile([128, 2 * QC], BF16, tag="t2", name=f"t2_{j}")
    with nc.allow_low_precision(reason="bf16 tree-sum of 8 A2A shards"):
        nc.vector.tensor_add(t0[:], rb4[:, 0, :], rb4[:, 1, :])
        nc.vector.tensor_add(t1[:], rb4[:, 2, :], rb4[:, 3, :])
        nc.vector.tensor_add(t0[:], t0[:], t1[:])
        nc.vector.tensor_add(t1[:], rb4[:, 4, :], rb4[:, 5, :])
        nc.vector.tensor_add(t2[:], rb4[:, 6, :], rb4[:, 7, :])
        nc.vector.tensor_add(t1[:], t1[:], t2[:])
        nc.vector.tensor_add(t0[:], t0[:], t1[:])
    nc.sync.dma_start(
        out_d.ap().rearrange("p (h ss) -> p h ss", h=2)[:, :, j * QC:(j + 1) * QC],
        t0[:].rearrange("p (h w) -> p h w", h=2))


_NC = None
_LAST_RESULTS = None  # BassKernelResults of the most recent run (for test harness)


def _causal_mask_tiles() -> np.ndarray:
    r = np.arange(128)[:, None]
    col = np.arange(QC)[None, :]
    tiles = [(col >= r + 128 * m).astype(np.float32) for m in range(4)]
    return np.concatenate(tiles, axis=1)  # [128, 2048]


def _wo2_input(Wo: np.ndarray, g: int) -> np.ndarray:
    """Pair-packed Wo stationary slices: [128, (pp*8 + ob)*128 + c] with
    wo2[r, ...] = Wo[(4g + 2pp + r//64)*64 + r%64, ob*128 + c]."""
    wo2 = np.empty((128, NP * 8 * 128), np.float32)
    for pp in range(NP):
        for half in range(2):
            h = 4 * g + 2 * pp + half
            blk = Wo[h * DH:(h + 1) * DH, :]  # [64, 1024]
            for ob in range(8):
                wo2[half * 64:(half + 1) * 64, (pp * 8 + ob) * 128:
                    (pp * 8 + ob + 1) * 128] = blk[:, ob * 128:(ob + 1) * 128]
    return wo2


def kernel(states, masks, Wq, Wk, Wv, Wo):
    global _NC, _LAST_RESULTS
    if _NC is None:
        _NC = _build()
    bf16 = ml_dtypes.bfloat16
    states = np.asarray(states, np.float32)
    Wq, Wk, Wv, Wo = (np.asarray(w, np.float32) for w in (Wq, Wk, Wv, Wo))
    cm = _causal_mask_tiles().astype(bf16)

    in_maps = []
    for c in range(NCORE):
        b, g = c // 4, c % 4
        cs = slice(g * 256, (g + 1) * 256)
        bsel = np.zeros((128, 2), np.float32)
        bsel[:, b] = 1.0
        in_maps.append({
            "statesT": np.ascontiguousarray(states[b].T).astype(bf16),
            "wq": np.ascontiguousarray(Wq[:, cs]).astype(bf16),
            "wk": np.ascontiguousarray(Wk[:, cs]).astype(bf16),
            "wv": np.ascontiguousarray(Wv[:, cs]).astype(bf16),
            "wo2": _wo2_input(Wo, g).astype(bf16),
            "cmask": cm,
            "bsel": bsel,
        })

    res = bass_utils.run_bass_kernel_spmd(_NC, in_maps, core_ids=list(range(NCORE)))
    _LAST_RESULTS = res
    # core c owns out^T rows [c*128,(c+1)*128) of each batch; stitch + transpose
    out = np.empty((B, S, D), np.float32)
    for c in range(NCORE):
        blk = np.asarray(res.results[c]["out"]).astype(np.float32)  # [128, 2*2048]
        for b in range(B):
            out[b][:, c * 128:(c + 1) * 128] = blk[:, b * S:(b + 1) * S].T
    return out
